# revision 6
# baseline (speedup 1.0000x reference)
"""Trainium2 Bass kernel: 1D box filter (window 17, zero-padded) along seq.

out[b, t, d] = (1/17) * sum_{i=-8..8} x[b, t+i, d]   (zero-padded in t)

Full input [8, 8192, 1024] f32. Batch dim sharded across 8 NeuronCores
(data-parallel, no cross-core communication).

The kernel is HBM-bandwidth bound, so all device I/O is float16: the host
casts the f32 input to f16 (rel rounding ~2^-11), the device computes the
window sum exactly in f32 PSUM via a banded matmul and stores f16, and the
host upcasts the result to f32. End-to-end L2 relative error ~5e-4 --
well inside the 2e-2 gate -- for half the HBM traffic of the f32 version
(~33.6 MB/core instead of ~72 MB/core).

Per-core algorithm: the window sum along seq is a banded matmul. Put 128
consecutive input seq rows on SBUF partitions (natural, fully-coalesced DMA
layout), multiply by a constant banded matrix A [K=128, M=112] with
A[k, m] = 1/17 for m <= k <= m+16, so PSUM[m, d] = window sum for output row
t0+m from input rows t0-8+k. 112 output rows per 128-row (halo +-8) input
tile; D=1024 split into two N=512 matmuls (PSUM bank limit). ScalarE and
VectorE split the PSUM -> SBUF f16 evacuation, DMA stores. Groups are
batched 4-at-a-time into supergroups (~1MB per HBM DMA, 5 SBUF bufs for
deep overlap); input DMAs ride the SP HWDGE ring, output DMAs the ACT ring
so stores never head-of-line-block loads.
"""

import numpy as np

import orjson

import concourse.bass as bass
import concourse.mybir as mybir
from concourse.bass_utils import run_bass_kernel_spmd
from concourse.tile import TileContext

# The installed walrus rejects >2 embedded sync waits on one instruction
# ("Too many sync wait commands"), while this Tile version freely packs 3+
# waits onto engine instructions (and every live semaphore onto the kernel
# tail drain). Post-process the serialized BIR: excess waits move onto
# standalone EventSemaphore instructions injected just before the owning
# instruction on the same engine queue, which preserves semantics (all
# waits still happen-before the instruction).
_WAIT_LIMIT_DEFAULT = 1
# EventSemaphore and Drain accept 2 embedded waits; LDWEIGHTS/DMA take 1.
_WAIT_LIMIT_BY_OPCODE = {"EventSemaphore": 2}
_EVSEM_WAITS = 2  # waits per injected EventSemaphore


def _split_sync_waits(bir_bytes: bytes) -> bytes:
    bir = orjson.loads(bir_bytes)
    ctr = 0
    for fn in bir.get("functions", []):
        for bb in fn.get("blocks", []):
            insts = bb.get("instructions")
            if not insts:
                continue
            out = []
            changed = False
            for ins in insts:
                si = ins.get("sync_info")
                ow = (si or {}).get("on_wait") or []
                limit = _WAIT_LIMIT_BY_OPCODE.get(
                    ins.get("opcode"), _WAIT_LIMIT_DEFAULT
                )
                if len(ow) > limit:
                    extra, keep = ow[:-limit] if limit else ow, ow[-limit:] if limit else []
                    for c0 in range(0, len(extra), _EVSEM_WAITS):
                        ctr += 1
                        out.append(
                            {
                                "debug": ins.get("debug", 0),
                                "engine": ins["engine"],
                                "ins": [],
                                "outs": [],
                                "name": f"wsplit-{ctr}-{ins['name']}",
                                "opcode": "EventSemaphore",
                                "sync_info": {
                                    "on_update": [],
                                    "on_wait": extra[c0 : c0 + _EVSEM_WAITS],
                                },
                            }
                        )
                    si["on_wait"] = keep
                    changed = True
                out.append(ins)
            if changed:
                bb["instructions"] = out
    return orjson.dumps(bir)


class WaitSplitBass(bass.Bass):
    def to_json_bytes(self) -> bytes:
        return _split_sync_waits(super().to_json_bytes())

W = 8            # half window
WIN = 2 * W + 1  # 17
S = 8192         # seq len per core
D = 1024         # feature dim
B = 8            # batch == number of cores
M = 112          # output rows per matmul group (128 - 2*W)
K = 128          # input rows per group (partition dim)
N_HALF = 512     # matmul moving free dim (one PSUM bank of fp32)

F32 = mybir.dt.float32
F16 = mybir.dt.float16


def make_band() -> np.ndarray:
    """A[k, m] = 1/17 if m <= k <= m+16 else 0, shape [128, 112] fp16."""
    a = np.zeros((K, M), dtype=np.float16)
    for m in range(M):
        a[m : m + WIN, m] = np.float16(1.0 / WIN)
    return a


def build_program(
    do_mm: bool = True,
    do_copy: bool = True,
    do_in: bool = True,
    do_out: bool = True,
    sg: int = 4,
    io_bufs: int = 5,
    out_dma_on_act: bool = True,
) -> bass.Bass:
    assert 72 % sg == 0
    nsg = 72 // sg
    nc = WaitSplitBass("TRN2", target_bir_lowering=False, debug=False)
    x = nc.dram_tensor("x", [S, D], F16, kind="ExternalInput")
    band = nc.dram_tensor("band", [K, M], F16, kind="ExternalInput")
    y = nc.dram_tensor("y", [S, D], F16, kind="ExternalOutput")

    with TileContext(nc) as tc:
        with (
            tc.tile_pool(name="const", bufs=1) as cpool,
            tc.tile_pool(name="io", bufs=io_bufs) as iopool,
            tc.tile_pool(name="psum", bufs=8, space="PSUM") as ppool,
        ):
            band_t = cpool.tile([K, M], F16)
            nc.sync.dma_start(out=band_t, in_=band.ap())

            def group(rhs2d, out_dst, m_rows, k_rows):
                # one 17-window group: 2 matmuls (d-halves) into separate
                # PSUM banks; evacuation split ScalarE/VectorE (real-HW
                # measured 1.6-1.8x faster than all-ScalarE evacuation)
                for h in range(2):
                    ps = ppool.tile([M, N_HALF], F32, tag="ps", name="ps")
                    if do_mm:
                        nc.tensor.matmul(
                            ps[:m_rows, :],
                            band_t[:k_rows, :m_rows],
                            rhs2d[:k_rows, h * N_HALF : (h + 1) * N_HALF],
                            start=True,
                            stop=True,
                        )
                    if do_copy:
                        dst = out_dst[:m_rows, h * N_HALF : (h + 1) * N_HALF]
                        if h == 0:
                            nc.scalar.copy(dst, ps[:m_rows, :])
                        else:
                            nc.vector.tensor_copy(out=dst, in_=ps[:m_rows, :])

            # ---- group 0: out rows [0, 112), input rows [-8, 120) ----
            g0_t = iopool.tile([K, D], F16, bufs=1)
            nc.any.memset(g0_t, 0.0)
            if do_in:
                nc.sync.dma_start(out=g0_t[W:K, :], in_=x.ap()[0 : K - W, :])
            g0_out = iopool.tile([M, D], F16, bufs=1)
            group(g0_t, g0_out, M, K)
            if do_out:
                nc.sync.dma_start(out=y.ap()[0:M, :], in_=g0_out)

            # ---- supergroups: groups 1..72, out rows [112, 8176) ----
            out_dma_eng = nc.scalar if out_dma_on_act else nc.sync
            for s in range(nsg):
                g0s = 1 + sg * s
                base_in = (M * g0s - W) * D
                in_sg = iopool.tile([K, sg, D], F16)
                if do_in:
                    nc.sync.dma_start(
                        out=in_sg,
                        in_=bass.AP(x, base_in, [[D, K], [M * D, sg], [1, D]]),
                    )
                out_sg = iopool.tile([M, sg, D], F16)
                for j in range(sg):
                    group(in_sg[:, j, :], out_sg[:, j, :], M, K)
                if do_out:
                    out_dma_eng.dma_start(
                        out=bass.AP(y, M * g0s * D, [[D, M], [M * D, sg], [1, D]]),
                        in_=out_sg,
                    )

            # ---- tail group: out rows [8176, 8192), input rows [8168, 8200) ----
            tail_rows = S - 73 * M           # 16
            tk = tail_rows + 2 * W           # 32 partitions
            tv = S - (73 * M - W)            # 24 valid input rows
            tail_t = iopool.tile([tk, D], F16, bufs=1)
            nc.any.memset(tail_t, 0.0)
            if do_in:
                nc.sync.dma_start(out=tail_t[0:tv, :], in_=x.ap()[S - tv : S, :])
            tail_out = iopool.tile([tail_rows, D], F16, bufs=1)
            group(tail_t, tail_out, tail_rows, tk)
            if do_out:
                nc.sync.dma_start(out=y.ap()[S - tail_rows : S, :], in_=tail_out)

    return nc


_CACHE: dict[str, bass.Bass] = {}


def get_program() -> bass.Bass:
    if "nc" not in _CACHE:
        _CACHE["nc"] = build_program()
    return _CACHE["nc"]


def make_in_maps(inputs: np.ndarray) -> list[dict[str, np.ndarray]]:
    band = make_band()
    x16 = np.ascontiguousarray(inputs).astype(np.float16)
    return [{"x": x16[b], "band": band} for b in range(B)]


def kernel(inputs) -> np.ndarray:
    inputs = np.asarray(inputs)
    assert inputs.shape == (B, S, D), inputs.shape
    nc = get_program()
    in_maps = make_in_maps(inputs)
    try:
        res = run_bass_kernel_spmd(nc, in_maps, list(range(B)))
    except Exception:
        # transient axon terminal failures have been observed; retry once
        res = run_bass_kernel_spmd(nc, in_maps, list(range(B)))
    return np.stack(
        [res.results[b]["y"].astype(np.float32) for b in range(B)], axis=0
    )



# revision 18
# speedup vs baseline: 1.0681x; 1.0681x over previous
"""Trainium2 Bass kernel: 1D box filter (window 17, zero-padded) along seq.

out[b, t, d] = (1/17) * sum_{i=-8..8} x[b, t+i, d]   (zero-padded in t)

Full input [8, 8192, 1024] f32. Batch dim sharded across 8 NeuronCores
(data-parallel, no cross-core communication).

The kernel is HBM-bandwidth bound, so all device I/O is float16: the host
casts the f32 input to f16 (rel rounding ~2^-11), the device computes the
window sum exactly in f32 PSUM via a banded matmul and stores f16, and the
host upcasts the result to f32. End-to-end L2 relative error ~5e-4 --
well inside the 2e-2 gate -- for half the HBM traffic of the f32 version
(~33.6 MB/core instead of ~72 MB/core).

Per-core algorithm: the window sum along seq is a banded matmul. Put 128
consecutive input seq rows on SBUF partitions (natural, fully-coalesced DMA
layout), multiply by a constant banded matrix A [K=128, M=112] with
A[k, m] = 1/17 for m <= k <= m+16, so PSUM[m, d] = window sum for output row
t0+m from input rows t0-8+k. 112 output rows per 128-row (halo +-8) input
tile; D=1024 split into two N=512 matmuls (PSUM bank limit). ScalarE and
VectorE split the PSUM -> SBUF f16 evacuation, DMA stores. Groups are
batched 4-at-a-time into supergroups (~1MB per HBM DMA, 5 SBUF bufs for
deep overlap); input DMAs ride the SP HWDGE ring, output DMAs the ACT ring
so stores never head-of-line-block loads.
"""

import numpy as np

import orjson

import concourse.bass as bass
import concourse.mybir as mybir
from concourse.bass_utils import run_bass_kernel_spmd
from concourse.tile import TileContext

# The installed walrus rejects >2 embedded sync waits on one instruction
# ("Too many sync wait commands"), while this Tile version freely packs 3+
# waits onto engine instructions (and every live semaphore onto the kernel
# tail drain). Post-process the serialized BIR: excess waits move onto
# standalone EventSemaphore instructions injected just before the owning
# instruction on the same engine queue, which preserves semantics (all
# waits still happen-before the instruction).
_WAIT_LIMIT_DEFAULT = 1
# EventSemaphore and Drain accept 2 embedded waits; LDWEIGHTS/DMA take 1.
_WAIT_LIMIT_BY_OPCODE = {"EventSemaphore": 2}
_EVSEM_WAITS = 2  # waits per injected EventSemaphore


def _split_sync_waits(bir_bytes: bytes) -> bytes:
    bir = orjson.loads(bir_bytes)
    ctr = 0
    for fn in bir.get("functions", []):
        for bb in fn.get("blocks", []):
            insts = bb.get("instructions")
            if not insts:
                continue
            out = []
            changed = False
            for ins in insts:
                si = ins.get("sync_info")
                ow = (si or {}).get("on_wait") or []
                limit = _WAIT_LIMIT_BY_OPCODE.get(
                    ins.get("opcode"), _WAIT_LIMIT_DEFAULT
                )
                if len(ow) > limit:
                    extra, keep = ow[:-limit] if limit else ow, ow[-limit:] if limit else []
                    for c0 in range(0, len(extra), _EVSEM_WAITS):
                        ctr += 1
                        out.append(
                            {
                                "debug": ins.get("debug", 0),
                                "engine": ins["engine"],
                                "ins": [],
                                "outs": [],
                                "name": f"wsplit-{ctr}-{ins['name']}",
                                "opcode": "EventSemaphore",
                                "sync_info": {
                                    "on_update": [],
                                    "on_wait": extra[c0 : c0 + _EVSEM_WAITS],
                                },
                            }
                        )
                    si["on_wait"] = keep
                    changed = True
                out.append(ins)
            if changed:
                bb["instructions"] = out
    return orjson.dumps(bir)


class WaitSplitBass(bass.Bass):
    def to_json_bytes(self) -> bytes:
        return _split_sync_waits(super().to_json_bytes())

W = 8            # half window
WIN = 2 * W + 1  # 17
S = 8192         # seq len per core
D = 1024         # feature dim
B = 8            # batch == number of cores
M = 112          # output rows per matmul group (128 - 2*W)
K = 128          # input rows per group (partition dim)
N_HALF = 512     # matmul moving free dim (one PSUM bank of fp32)

F32 = mybir.dt.float32
F16 = mybir.dt.float16


def make_band() -> np.ndarray:
    """A[k, m] = 1/17 if m <= k <= m+16 else 0, shape [128, 112] fp16."""
    a = np.zeros((K, M), dtype=np.float16)
    for m in range(M):
        a[m : m + WIN, m] = np.float16(1.0 / WIN)
    return a


def make_bands() -> dict[str, np.ndarray]:
    """Packed band constants, all used at base partition 0.

    Engine and PE access-pattern start partitions must be multiples of 32
    (PE matmul operands: 0/32/64 only), so each group reads its 112 fresh
    input rows at base 0 and its 16 halo rows from a separate tile at base
    0, and the window sum is two PSUM-accumulated matmuls:

      cols [0,112):  band_main = A[16:128] -- weights of the fresh rows
      cols [112,128): band_halo = A[0:16, 0:16] -- weights of the halo rows
                      (halo rows only contribute to out rows [0,16))
    """
    a = make_band()
    pack = np.zeros((K, K), dtype=np.float16)
    pack[:M, :M] = a[2 * W :]
    pack[: 2 * W, M : M + 2 * W] = a[: 2 * W, : 2 * W]
    return {"bands": np.ascontiguousarray(pack)}


def build_program(
    do_mm: bool = True,
    do_copy: bool = True,
    do_in: bool = True,
    do_out: bool = True,
    sg: int = 4,
    io_bufs: int = 8,
    out_dma_on_act: bool = True,
) -> bass.Bass:
    """Halo-free input streaming: every input row is DMA'd from HBM exactly
    once. All 74 output groups are uniform: group g covers out rows
    [112g, 112g+112) (the last only 16), computed from 112 "fresh" input
    rows [112g+8, 112g+120) at partitions [0,112) of its supergroup tile
    column plus 16 "halo" rows [112g-8, 112g+8) in a separate [16, D] tile,
    staged by a cheap engine copy from the previous column's partitions
    [96,112) (an allowed mod-32 base). The first group's halo (8 zero-pad
    rows + input rows [0,8)) is memset + a tiny DMA. Each group is two
    PSUM-accumulated matmuls per d-half (K=112 band_main, K=16 band_halo).
    Halo copies rotate over GpSimd/DVE/GpSimd/ACT; PSUM evacuation is
    split ScalarE/VectorE; input DMAs ride the SP HWDGE ring, output DMAs
    the ACT ring. The final two groups use per-column DMAs on the
    then-idle SP ring so the drain only serializes one short chain.
    """
    assert 72 % sg == 0
    nsg = 72 // sg                   # full supergroups (groups 0..71)
    HB = 2 * W                       # halo rows (16)
    nc = WaitSplitBass("TRN2", target_bir_lowering=False, debug=False)
    x = nc.dram_tensor("x", [S, D], F16, kind="ExternalInput")
    bands = nc.dram_tensor("bands", [K, K], F16, kind="ExternalInput")
    y = nc.dram_tensor("y", [S, D], F16, kind="ExternalOutput")

    with TileContext(nc) as tc:
        with (
            tc.tile_pool(name="const", bufs=1) as cpool,
            tc.tile_pool(name="io", bufs=io_bufs) as iopool,
            tc.tile_pool(name="psum", bufs=8, space="PSUM") as ppool,
        ):
            bands_t = cpool.tile([K, K], F16)
            nc.sync.dma_start(out=bands_t, in_=bands.ap())
            band_main = bands_t[:, :M]
            band_halo = bands_t[:HB, M : M + HB]

            out_dma_eng = nc.scalar if out_dma_on_act else nc.sync

            def group(main_rhs, main_k, halo_rhs, out_dst, m_rows):
                # window sum = K=main_k matmul (fresh rows) + K=16 matmul
                # (halo rows, contributes to out rows [0,16) only),
                # accumulated in one PSUM bank; evacuation split
                # ScalarE/VectorE (real-HW measured 1.6-1.8x faster than
                # all-ScalarE evacuation)
                for h in range(2):
                    ps = ppool.tile([M, N_HALF], F32, tag="ps", name="ps")
                    if do_mm:
                        nc.tensor.matmul(
                            ps[:m_rows, :],
                            band_main[:main_k, :m_rows],
                            main_rhs[:, h * N_HALF : (h + 1) * N_HALF],
                            start=True,
                            stop=False,
                        )
                        nc.tensor.matmul(
                            ps[:HB, :] if m_rows >= HB else ps[:m_rows, :],
                            band_halo[:, : min(HB, m_rows)],
                            halo_rhs[:, h * N_HALF : (h + 1) * N_HALF],
                            start=False,
                            stop=True,
                        )
                    if do_copy:
                        dst = out_dst[:m_rows, h * N_HALF : (h + 1) * N_HALF]
                        if h == 0:
                            nc.scalar.copy(dst, ps[:m_rows, :])
                        else:
                            nc.vector.tensor_copy(out=dst, in_=ps[:m_rows, :])

            def halo_copy(g, dst, src):
                # halo staging: DVE SBUF->SBUF f16 copies run in 4x mode
                # (~330ns); GpSimd takes the other half (~1.6us, idle engine).
                # ACT is kept free for PSUM evacuation + out-DMA issue so the
                # compute drain never paces the output stream.
                if g % 2 == 0:
                    nc.vector.tensor_copy(out=dst, in_=src)
                else:
                    nc.gpsimd.tensor_copy(out=dst, in_=src)

            # halo of group 0: 8 zero-pad rows, then input rows [0, 8)
            halo0 = iopool.tile([HB, D], F16, bufs=1)
            nc.any.memset(halo0, 0.0)
            if do_in:
                nc.sync.dma_start(out=halo0[W:HB, :], in_=x.ap()[0:W, :])

            # ---- full supergroups: groups 0..71 ----
            # in_sg and out_sg share one buffer rotation (same tag/shape):
            # the input DMA of supergroup s+bufs/2 WARs on supergroup s's
            # tile, so the input stream cannot run unboundedly ahead of the
            # output stream -- keeps the DMA engine in/out interleaved to
            # the end instead of piling compute-gated stores into the drain.
            halo_prev_src = None        # previous column's partitions [96,112)
            for s in range(nsg):
                in_sg = iopool.tile([M, sg, D], F16, tag="io", name="in_sg")
                if do_in:
                    nc.sync.dma_start(
                        out=in_sg,
                        in_=bass.AP(
                            x, (M * sg * s + W) * D, [[D, M], [M * D, sg], [1, D]]
                        ),
                    )
                out_sg = iopool.tile([M, sg, D], F16, tag="io", name="out_sg")
                for j in range(sg):
                    g = sg * s + j
                    if g == 0:
                        halo = halo0
                    else:
                        src = (
                            halo_prev_src
                            if j == 0
                            else in_sg[M - HB : M, j - 1, :]
                        )
                        halo = iopool.tile([HB, D], F16, tag="halo", bufs=6)
                        halo_copy(g, halo, src)
                    group(in_sg[:, j, :], M, halo, out_sg[:, j, :], M)
                halo_prev_src = in_sg[M - HB : M, sg - 1, :]
                if do_out:
                    out_dma_eng.dma_start(
                        out=bass.AP(y, M * sg * s * D, [[D, M], [M * D, sg], [1, D]]),
                        in_=out_sg,
                    )

            # ---- final groups 72, 73: per-column DMAs on the idle SP ring
            # so the drain only serializes one short chain ----
            g72_in = iopool.tile([M, D], F16, bufs=1)
            if do_in:
                nc.sync.dma_start(out=g72_in, in_=x.ap()[M * 72 + W : M * 73 + W, :])
            tail_rows = S - 73 * M       # 16
            g73_in = iopool.tile([W, D], F16, bufs=1)
            if do_in:
                nc.sync.dma_start(out=g73_in, in_=x.ap()[M * 73 + W : S, :])

            g72_halo = iopool.tile([HB, D], F16, bufs=1)
            halo_copy(72, g72_halo, halo_prev_src)
            g72_out = iopool.tile([M, D], F16, bufs=1)
            group(g72_in, M, g72_halo, g72_out, M)
            if do_out:
                nc.sync.dma_start(out=y.ap()[M * 72 : M * 73, :], in_=g72_out)

            g73_halo = iopool.tile([HB, D], F16, bufs=1)
            halo_copy(73, g73_halo, g72_in[M - HB : M, :])
            g73_out = iopool.tile([tail_rows, D], F16, bufs=1)
            group(g73_in, W, g73_halo, g73_out, tail_rows)
            if do_out:
                nc.sync.dma_start(out=y.ap()[73 * M : S, :], in_=g73_out)

    return nc


_CACHE: dict[str, bass.Bass] = {}


def get_program() -> bass.Bass:
    if "nc" not in _CACHE:
        _CACHE["nc"] = build_program()
    return _CACHE["nc"]


def make_in_maps(inputs: np.ndarray) -> list[dict[str, np.ndarray]]:
    bands = make_bands()
    x16 = np.ascontiguousarray(inputs).astype(np.float16)
    return [{"x": x16[b], **bands} for b in range(B)]


def kernel(inputs) -> np.ndarray:
    inputs = np.asarray(inputs)
    assert inputs.shape == (B, S, D), inputs.shape
    nc = get_program()
    in_maps = make_in_maps(inputs)
    try:
        res = run_bass_kernel_spmd(nc, in_maps, list(range(B)))
    except Exception:
        # transient axon terminal failures have been observed; retry once
        res = run_bass_kernel_spmd(nc, in_maps, list(range(B)))
    return np.stack(
        [res.results[b]["y"].astype(np.float32) for b in range(B)], axis=0
    )



# revision 24
# speedup vs baseline: 1.0683x; 1.0002x over previous
"""Trainium2 Bass kernel: 1D box filter (window 17, zero-padded) along seq.

out[b, t, d] = (1/17) * sum_{i=-8..8} x[b, t+i, d]   (zero-padded in t)

Full input [8, 8192, 1024] f32. Batch dim sharded across 8 NeuronCores
(data-parallel, no cross-core communication).

The kernel is HBM-bandwidth bound, so all device I/O is float16: the host
casts the f32 input to f16 (rel rounding ~2^-11), the device computes the
window sum exactly in f32 PSUM via a banded matmul and stores f16, and the
host upcasts the result to f32. End-to-end L2 relative error ~5e-4 --
well inside the 2e-2 gate -- for half the HBM traffic of the f32 version
(~33.6 MB/core instead of ~72 MB/core).

Per-core algorithm: the window sum along seq is a banded matmul. Put 128
consecutive input seq rows on SBUF partitions (natural, fully-coalesced DMA
layout), multiply by a constant banded matrix A [K=128, M=112] with
A[k, m] = 1/17 for m <= k <= m+16, so PSUM[m, d] = window sum for output row
t0+m from input rows t0-8+k. 112 output rows per 128-row (halo +-8) input
tile; D=1024 split into two N=512 matmuls (PSUM bank limit). ScalarE and
VectorE split the PSUM -> SBUF f16 evacuation, DMA stores. Groups are
batched 4-at-a-time into supergroups (~1MB per HBM DMA, 5 SBUF bufs for
deep overlap); input DMAs ride the SP HWDGE ring, output DMAs the ACT ring
so stores never head-of-line-block loads.
"""

import numpy as np

import orjson

import concourse.bass as bass
import concourse.mybir as mybir
from concourse.bass_utils import run_bass_kernel_spmd
from concourse.tile import TileContext

# The installed walrus rejects >2 embedded sync waits on one instruction
# ("Too many sync wait commands"), while this Tile version freely packs 3+
# waits onto engine instructions (and every live semaphore onto the kernel
# tail drain). Post-process the serialized BIR: excess waits move onto
# standalone EventSemaphore instructions injected just before the owning
# instruction on the same engine queue, which preserves semantics (all
# waits still happen-before the instruction).
_WAIT_LIMIT_DEFAULT = 1
# EventSemaphore and Drain accept 2 embedded waits; LDWEIGHTS/DMA take 1.
_WAIT_LIMIT_BY_OPCODE = {"EventSemaphore": 2}
_EVSEM_WAITS = 2  # waits per injected EventSemaphore


def _split_sync_waits(bir_bytes: bytes) -> bytes:
    bir = orjson.loads(bir_bytes)
    ctr = 0
    for fn in bir.get("functions", []):
        for bb in fn.get("blocks", []):
            insts = bb.get("instructions")
            if not insts:
                continue
            out = []
            changed = False
            for ins in insts:
                si = ins.get("sync_info")
                ow = (si or {}).get("on_wait") or []
                limit = _WAIT_LIMIT_BY_OPCODE.get(
                    ins.get("opcode"), _WAIT_LIMIT_DEFAULT
                )
                if len(ow) > limit:
                    extra, keep = ow[:-limit] if limit else ow, ow[-limit:] if limit else []
                    for c0 in range(0, len(extra), _EVSEM_WAITS):
                        ctr += 1
                        out.append(
                            {
                                "debug": ins.get("debug", 0),
                                "engine": ins["engine"],
                                "ins": [],
                                "outs": [],
                                "name": f"wsplit-{ctr}-{ins['name']}",
                                "opcode": "EventSemaphore",
                                "sync_info": {
                                    "on_update": [],
                                    "on_wait": extra[c0 : c0 + _EVSEM_WAITS],
                                },
                            }
                        )
                    si["on_wait"] = keep
                    changed = True
                out.append(ins)
            if changed:
                bb["instructions"] = out
    return orjson.dumps(bir)


class WaitSplitBass(bass.Bass):
    def to_json_bytes(self) -> bytes:
        return _split_sync_waits(super().to_json_bytes())

W = 8            # half window
WIN = 2 * W + 1  # 17
S = 8192         # seq len per core
D = 1024         # feature dim
B = 8            # batch == number of cores
M = 112          # output rows per matmul group (128 - 2*W)
K = 128          # input rows per group (partition dim)
N_HALF = 512     # matmul moving free dim (one PSUM bank of fp32)

F32 = mybir.dt.float32
F16 = mybir.dt.float16


def make_band() -> np.ndarray:
    """A[k, m] = 1/17 if m <= k <= m+16 else 0, shape [128, 112] fp16."""
    a = np.zeros((K, M), dtype=np.float16)
    for m in range(M):
        a[m : m + WIN, m] = np.float16(1.0 / WIN)
    return a


def make_bands() -> dict[str, np.ndarray]:
    """Packed band constants, all used at base partition 0.

    Engine and PE access-pattern start partitions must be multiples of 32
    (PE matmul operands: 0/32/64 only), so each group reads its 112 fresh
    input rows at base 0 and its 16 halo rows from a separate tile at base
    0, and the window sum is two PSUM-accumulated matmuls:

      cols [0,112):  band_main = A[16:128] -- weights of the fresh rows
      cols [112,128): band_halo = A[0:16, 0:16] -- weights of the halo rows
                      (halo rows only contribute to out rows [0,16))
    """
    a = make_band()
    pack = np.zeros((K, K), dtype=np.float16)
    pack[:M, :M] = a[2 * W :]
    pack[: 2 * W, M : M + 2 * W] = a[: 2 * W, : 2 * W]
    return {"bands": np.ascontiguousarray(pack)}


def build_program(
    do_mm: bool = True,
    do_copy: bool = True,
    do_in: bool = True,
    do_out: bool = True,
    sg: int = 8,
    io_bufs: int = 8,
    out_dma_on_act: bool = True,
) -> bass.Bass:
    """Halo-free input streaming: every input row is DMA'd from HBM exactly
    once. All 74 output groups are uniform: group g covers out rows
    [112g, 112g+112) (the last only 16), computed from 112 "fresh" input
    rows [112g+8, 112g+120) at partitions [0,112) of its supergroup tile
    column plus 16 "halo" rows [112g-8, 112g+8) in a separate [16, D] tile,
    staged by a cheap engine copy from the previous column's partitions
    [96,112) (an allowed mod-32 base). The first group's halo (8 zero-pad
    rows + input rows [0,8)) is memset + a tiny DMA. Each group is two
    PSUM-accumulated matmuls per d-half (K=112 band_main, K=16 band_halo).
    Halo copies rotate over GpSimd/DVE/GpSimd/ACT; PSUM evacuation is
    split ScalarE/VectorE; input DMAs ride the SP HWDGE ring, output DMAs
    the ACT ring. The final two groups use per-column DMAs on the
    then-idle SP ring so the drain only serializes one short chain.
    """
    assert 72 % sg == 0
    nsg = 72 // sg                   # full supergroups (groups 0..71)
    HB = 2 * W                       # halo rows (16)
    nc = WaitSplitBass("TRN2", target_bir_lowering=False, debug=False)
    x = nc.dram_tensor("x", [S, D], F16, kind="ExternalInput")
    bands = nc.dram_tensor("bands", [K, K], F16, kind="ExternalInput")
    y = nc.dram_tensor("y", [S, D], F16, kind="ExternalOutput")

    with TileContext(nc) as tc:
        with (
            tc.tile_pool(name="const", bufs=1) as cpool,
            tc.tile_pool(name="io", bufs=io_bufs) as iopool,
            tc.tile_pool(name="psum", bufs=8, space="PSUM") as ppool,
        ):
            bands_t = cpool.tile([K, K], F16)
            nc.sync.dma_start(out=bands_t, in_=bands.ap())
            band_main = bands_t[:, :M]
            band_halo = bands_t[:HB, M : M + HB]

            out_dma_eng = nc.scalar if out_dma_on_act else nc.sync

            def group(main_rhs, main_k, halo_rhs, out_dst, m_rows):
                # window sum = K=main_k matmul (fresh rows) + K=16 matmul
                # (halo rows, contributes to out rows [0,16) only),
                # accumulated in one PSUM bank; evacuation split
                # ScalarE/VectorE (real-HW measured 1.6-1.8x faster than
                # all-ScalarE evacuation)
                for h in range(2):
                    ps = ppool.tile([M, N_HALF], F32, tag="ps", name="ps")
                    if do_mm:
                        nc.tensor.matmul(
                            ps[:m_rows, :],
                            band_main[:main_k, :m_rows],
                            main_rhs[:, h * N_HALF : (h + 1) * N_HALF],
                            start=True,
                            stop=False,
                        )
                        nc.tensor.matmul(
                            ps[:HB, :] if m_rows >= HB else ps[:m_rows, :],
                            band_halo[:, : min(HB, m_rows)],
                            halo_rhs[:, h * N_HALF : (h + 1) * N_HALF],
                            start=False,
                            stop=True,
                        )
                    if do_copy:
                        dst = out_dst[:m_rows, h * N_HALF : (h + 1) * N_HALF]
                        if h == 0:
                            nc.scalar.copy(dst, ps[:m_rows, :])
                        else:
                            nc.vector.tensor_copy(out=dst, in_=ps[:m_rows, :])

            def halo_copy(g, dst, src):
                # halo staging: DVE SBUF->SBUF f16 copies run in 4x mode
                # (~330ns); GpSimd takes the other half (~1.6us, idle engine).
                # ACT is kept free for PSUM evacuation + out-DMA issue so the
                # compute drain never paces the output stream.
                if g % 2 == 0:
                    nc.vector.tensor_copy(out=dst, in_=src)
                else:
                    nc.gpsimd.tensor_copy(out=dst, in_=src)

            # halo of group 0: 8 zero-pad rows, then input rows [0, 8)
            halo0 = iopool.tile([HB, D], F16, bufs=1)
            nc.any.memset(halo0, 0.0)
            if do_in:
                nc.sync.dma_start(out=halo0[W:HB, :], in_=x.ap()[0:W, :])

            # ---- full supergroups: groups 0..71 ----
            # in_sg and out_sg share one buffer rotation (same tag/shape):
            # the input DMA of supergroup s+bufs/2 WARs on supergroup s's
            # tile, so the input stream cannot run unboundedly ahead of the
            # output stream -- keeps the DMA engine in/out interleaved to
            # the end instead of piling compute-gated stores into the drain.
            halo_prev_src = None        # previous column's partitions [96,112)
            for s in range(nsg):
                in_sg = iopool.tile([M, sg, D], F16, tag="io", name="in_sg")
                if do_in:
                    nc.sync.dma_start(
                        out=in_sg,
                        in_=bass.AP(
                            x, (M * sg * s + W) * D, [[D, M], [M * D, sg], [1, D]]
                        ),
                    )
                out_sg = iopool.tile([M, sg, D], F16, tag="io", name="out_sg")
                for j in range(sg):
                    g = sg * s + j
                    if g == 0:
                        halo = halo0
                    else:
                        src = (
                            halo_prev_src
                            if j == 0
                            else in_sg[M - HB : M, j - 1, :]
                        )
                        halo = iopool.tile([HB, D], F16, tag="halo", bufs=6)
                        halo_copy(g, halo, src)
                    group(in_sg[:, j, :], M, halo, out_sg[:, j, :], M)
                halo_prev_src = in_sg[M - HB : M, sg - 1, :]
                if do_out:
                    out_dma_eng.dma_start(
                        out=bass.AP(y, M * sg * s * D, [[D, M], [M * D, sg], [1, D]]),
                        in_=out_sg,
                    )

            # ---- final groups 72, 73: per-column DMAs on the idle SP ring
            # so the drain only serializes one short chain ----
            g72_in = iopool.tile([M, D], F16, bufs=1)
            if do_in:
                nc.sync.dma_start(out=g72_in, in_=x.ap()[M * 72 + W : M * 73 + W, :])
            tail_rows = S - 73 * M       # 16
            g73_in = iopool.tile([W, D], F16, bufs=1)
            if do_in:
                nc.sync.dma_start(out=g73_in, in_=x.ap()[M * 73 + W : S, :])

            g72_halo = iopool.tile([HB, D], F16, bufs=1)
            halo_copy(72, g72_halo, halo_prev_src)
            g72_out = iopool.tile([M, D], F16, bufs=1)
            group(g72_in, M, g72_halo, g72_out, M)
            if do_out:
                nc.sync.dma_start(out=y.ap()[M * 72 : M * 73, :], in_=g72_out)

            g73_halo = iopool.tile([HB, D], F16, bufs=1)
            halo_copy(73, g73_halo, g72_in[M - HB : M, :])
            g73_out = iopool.tile([tail_rows, D], F16, bufs=1)
            group(g73_in, W, g73_halo, g73_out, tail_rows)
            if do_out:
                nc.sync.dma_start(out=y.ap()[73 * M : S, :], in_=g73_out)

    return nc


_CACHE: dict[str, bass.Bass] = {}


def get_program() -> bass.Bass:
    if "nc" not in _CACHE:
        _CACHE["nc"] = build_program()
    return _CACHE["nc"]


def make_in_maps(inputs: np.ndarray) -> list[dict[str, np.ndarray]]:
    bands = make_bands()
    x16 = np.ascontiguousarray(inputs).astype(np.float16)
    return [{"x": x16[b], **bands} for b in range(B)]


def kernel(inputs) -> np.ndarray:
    inputs = np.asarray(inputs)
    assert inputs.shape == (B, S, D), inputs.shape
    nc = get_program()
    in_maps = make_in_maps(inputs)
    try:
        res = run_bass_kernel_spmd(nc, in_maps, list(range(B)))
    except Exception:
        # transient axon terminal failures have been observed; retry once
        res = run_bass_kernel_spmd(nc, in_maps, list(range(B)))
    return np.stack(
        [res.results[b]["y"].astype(np.float32) for b in range(B)], axis=0
    )

